# revision 5
# baseline (speedup 1.0000x reference)
"""Causal self-attention (B=4, T=2048, C=1024, H=16, D=64) on 8 NeuronCores.

Sharding: core = (batch b, head-group g) with b = core//2, g = core%2.
Each core computes its batch's attention for 8 heads (g picks heads 8g..8g+7)
plus the corresponding slice of the QKV/output projections (tensor parallel,
column/row split).  The output projection is row-parallel, so the full output
for batch b is the SUM of the two partial outputs of cores (2b, 2b+1); that
reduction is done on the host during the gather/unshard step.

Device kernel strategy (per core):
  - x^T materialized chunk-by-chunk via PE transposes (fp32 has no DMA
    transpose).
  - q^T, k^T computed as W^T @ x^T (so no transpose of activations needed);
    v computed in natural [T, D] layout as x @ Wv.  1/sqrt(D) is folded into
    Wq/bq on the host (exact: power of two).
  - scores are computed TRANSPOSED (k-position on partitions) so that the
    probs @ v contraction needs no transpose;  softmax runs without max
    subtraction (scores are bounded ~|2|, exp is safe) and the denominator
    comes free as a 65th "ones" column in the PV matmul.
  - causality by restricting matmul column ranges per k-tile + one 128x128
    triangle mask multiply per diagonal block.
  - all matmuls in float32r (full PE rate at free-dim>=256; ~11-bit mantissa
    rounding of inputs, accumulation in fp32).
"""

import os
import tempfile
from contextlib import ExitStack

import numpy as np

import concourse.bass as bass
import concourse.mybir as mybir
import concourse.tile as tile
from concourse import bacc
from concourse.bass_utils import run_bass_kernel_spmd
from concourse.masks import make_identity, make_upper_triangular

B, T_FULL, C, H, D = 4, 2048, 1024, 16, 64
HG = 2                # head-group (tensor-parallel) factor
GH = H // HG          # heads per core = 8
F = GH * D            # per-core projection width = 512
N_CORES = B * HG      # 8
FP = mybir.dt.float32
FPR = mybir.dt.float32r


def _emit(nc: bass.Bass, T: int):
    CH = T // 512            # 512-row query chunks
    KO = C // 128            # contraction subtiles for C (8)
    FT = F // 128            # feature tiles (4)
    AF = mybir.ActivationFunctionType

    x = nc.dram_tensor("x", [T, C], FP, kind="ExternalInput").ap()
    wq = nc.dram_tensor("wq", [C, F], FPR, kind="ExternalInput").ap()
    wk = nc.dram_tensor("wk", [C, F], FPR, kind="ExternalInput").ap()
    wv = nc.dram_tensor("wv", [C, F], FPR, kind="ExternalInput").ap()
    bq = nc.dram_tensor("bq", [F], FP, kind="ExternalInput").ap()
    bk = nc.dram_tensor("bk", [F], FP, kind="ExternalInput").ap()
    bv = nc.dram_tensor("bv", [F], FP, kind="ExternalInput").ap()
    wo = nc.dram_tensor("wo", [F, C], FPR, kind="ExternalInput").ap()
    bo = nc.dram_tensor("bo", [C], FP, kind="ExternalInput").ap()
    y = nc.dram_tensor("y", [T, C], FP, kind="ExternalOutput").ap()

    with tile.TileContext(nc) as tc, ExitStack() as ctx:
        const = ctx.enter_context(tc.tile_pool(name="const", bufs=1))
        pers = ctx.enter_context(tc.tile_pool(name="pers", bufs=1))
        xp = ctx.enter_context(tc.tile_pool(name="xp", bufs=4))
        xtp = ctx.enter_context(tc.tile_pool(name="xtp", bufs=1))
        qtp = ctx.enter_context(tc.tile_pool(name="qtp", bufs=1))
        ptp = ctx.enter_context(tc.tile_pool(name="ptp", bufs=2))
        otp = ctx.enter_context(tc.tile_pool(name="otp", bufs=4))
        yp = ctx.enter_context(tc.tile_pool(name="yp", bufs=2))
        smp = ctx.enter_context(tc.tile_pool(name="smp", bufs=2))
        ps_big = ctx.enter_context(tc.tile_pool(name="ps_big", bufs=2, space="PSUM"))
        ps_pv = ctx.enter_context(tc.tile_pool(name="ps_pv", bufs=2, space="PSUM"))
        ps_sm = ctx.enter_context(tc.tile_pool(name="ps_sm", bufs=2, space="PSUM"))

        # ---- constants / weights -------------------------------------------------
        ident = const.tile([128, 128], FP)
        make_identity(nc, ident[:])
        tri = const.tile([128, 128], FP)        # tri[r, c] = 1.0 if c >= r else 0
        make_upper_triangular(nc, tri[:], val=1.0, diag=True)
        ones128 = const.tile([128, 128], FP)
        nc.gpsimd.memset(ones128[:], 1.0)

        wq_sb = const.tile([128, KO, F], FPR)
        nc.sync.dma_start(wq_sb[:], wq.rearrange("(ko p) f -> p ko f", p=128))
        wk_sb = const.tile([128, KO, F], FPR)
        nc.sync.dma_start(wk_sb[:], wk.rearrange("(ko p) f -> p ko f", p=128))
        wv_sb = const.tile([128, KO, F], FPR)
        nc.sync.dma_start(wv_sb[:], wv.rearrange("(ko p) f -> p ko f", p=128))
        wo_sb = const.tile([128, FT, C], FPR)
        nc.sync.dma_start(wo_sb[:], wo.rearrange("(ft p) c -> p ft c", p=128))

        bq_sb = const.tile([128, FT], FP)
        nc.sync.dma_start(bq_sb[:], bq.rearrange("(ft p) -> p ft", p=128))
        bk_sb = const.tile([128, FT], FP)
        nc.sync.dma_start(bk_sb[:], bk.rearrange("(ft p) -> p ft", p=128))

        bv_bc = const.tile([128, F], FP)
        nc.sync.dma_start(bv_bc[0:1, :], bv.rearrange("(o f) -> o f", o=1))
        nc.gpsimd.partition_broadcast(bv_bc[:], bv_bc[0:1, :])
        bo_bc = const.tile([128, C], FP)
        nc.sync.dma_start(bo_bc[0:1, :], bo.rearrange("(o c) -> o c", o=1))
        nc.gpsimd.partition_broadcast(bo_bc[:], bo_bc[0:1, :])

        # ---- persistent k^T / v -------------------------------------------------
        # k^T head-pair packed: head h lives at partitions 64*(h%2)..+64, pair h//2
        kt_sb = pers.tile([128, FT, T], FPR)
        # v natural, per k-tile, per head, 64 features + ones column
        v_sb = pers.tile([128, T // 128, GH, D + 1], FPR)
        nc.vector.tensor_copy(
            v_sb[:, :, :, D : D + 1],
            ones128[:, 0 : (T // 128) * GH].rearrange("p (k h o) -> p k h o", k=T // 128, o=1),
        )

        for c in range(CH):
            # ---- A: x^T for this chunk ------------------------------------------
            xnat = [xp.tile([128, C], FP, tag="xnat", name=f"xnat_{c}_{i}") for i in range(4)]
            for s in range(4):
                nc.sync.dma_start(xnat[s][:], x[c * 512 + s * 128 : c * 512 + (s + 1) * 128, :])
            xt_sb = xtp.tile([128, KO, 512], FPR)
            for kop in range(KO // 2):
                ps_t = ps_big.tile([128, 1024], FP, tag="ps_big")
                for u in range(2):
                    ko = kop * 2 + u
                    for s in range(4):
                        nc.tensor.transpose(
                            ps_t[:, u * 512 + s * 128 : u * 512 + (s + 1) * 128],
                            xnat[s][:, ko * 128 : (ko + 1) * 128],
                            ident[:],
                        )
                nc.vector.tensor_copy(xt_sb[:, kop * 2 : kop * 2 + 2, :], ps_t[:].rearrange("p (u t) -> p u t", u=2))

            # ---- B: q^T, k^T, v projections -------------------------------------
            qt_sb = qtp.tile([128, FT, 512], FPR)
            for ft in range(FT):
                ps_q = ps_sm.tile([128, 512], FP, tag="ps_sm")
                for ko in range(KO):
                    nc.tensor.matmul(
                        ps_q[:],
                        wq_sb[:, ko, ft * 128 : (ft + 1) * 128],
                        xt_sb[:, ko, :],
                        start=(ko == 0),
                        stop=(ko == KO - 1),
                    )
                nc.vector.tensor_scalar_add(qt_sb[:, ft, :], ps_q[:], bq_sb[:, ft : ft + 1])
            for ft in range(FT):
                ps_k = ps_sm.tile([128, 512], FP, tag="ps_sm")
                for ko in range(KO):
                    nc.tensor.matmul(
                        ps_k[:],
                        wk_sb[:, ko, ft * 128 : (ft + 1) * 128],
                        xt_sb[:, ko, :],
                        start=(ko == 0),
                        stop=(ko == KO - 1),
                    )
                nc.vector.tensor_scalar_add(
                    kt_sb[:, ft, c * 512 : (c + 1) * 512], ps_k[:], bk_sb[:, ft : ft + 1]
                )
            for s in range(4):
                ps_v = ps_sm.tile([128, 512], FP, tag="ps_sm")
                for ko in range(KO):
                    nc.tensor.matmul(
                        ps_v[:],
                        xt_sb[:, ko, s * 128 : (s + 1) * 128],
                        wv_sb[:, ko, :],
                        start=(ko == 0),
                        stop=(ko == KO - 1),
                    )
                nc.vector.tensor_tensor(
                    v_sb[:, c * 4 + s, :, 0:D],
                    ps_v[:].rearrange("p (h d) -> p h d", h=GH),
                    bv_bc[:].rearrange("p (h d) -> p h d", h=GH),
                    mybir.AluOpType.add,
                )

            # ---- C: attention ----------------------------------------------------
            KT = 4 * (c + 1)
            ot = [otp.tile([128, 512], FPR, tag="ot", name=f"ot_{c}_{i}") for i in range(FT)]
            for h in range(GH):
                hp, po = h // 2, 64 * (h % 2)
                ps_pv_t = ps_pv.tile([65, 512], FP, tag="ps_pv")
                for jj in range(0, KT, 2):
                    ps_s = ps_big.tile([128, 1024], FP, tag="ps_big")
                    pt_t = ptp.tile([128, 1024], FPR, tag="pt")
                    regions = []
                    for u in range(2):
                        j = jj + u
                        if j >= KT:
                            break
                        off = max(0, (j - 4 * c) * 128)
                        regions.append((u, j, off))
                        nc.tensor.matmul(
                            ps_s[:, u * 512 + off : (u + 1) * 512],
                            kt_sb[po : po + 64, hp, j * 128 : (j + 1) * 128],
                            qt_sb[po : po + 64, hp, off:512],
                            start=True,
                            stop=True,
                        )
                    if len(regions) == 2 and regions[0][2] == 0 and regions[1][2] == 0:
                        nc.scalar.activation(pt_t[:], ps_s[:], AF.Exp)
                    else:
                        for u, j, off in regions:
                            nc.scalar.activation(
                                pt_t[:, u * 512 + off : (u + 1) * 512],
                                ps_s[:, u * 512 + off : (u + 1) * 512],
                                AF.Exp,
                            )
                    for u, j, off in regions:
                        if j >= 4 * c:  # diagonal block: strict lower triangle -> 0
                            blk = slice(u * 512 + off, u * 512 + off + 128)
                            nc.vector.tensor_mul(pt_t[:, blk], pt_t[:, blk], tri[:])
                    for u, j, off in regions:
                        nc.tensor.matmul(
                            ps_pv_t[:, off:512],
                            v_sb[:, j, h, :],
                            pt_t[:, u * 512 + off : (u + 1) * 512],
                            start=(j == 0),
                            stop=(j == KT - 1),
                        )
                r_t = smp.tile([1, 512], FP, tag="r")
                nc.vector.reciprocal(r_t[:], ps_pv_t[64:65, :])
                rb_t = smp.tile([64, 512], FP, tag="rb")
                nc.gpsimd.partition_broadcast(rb_t[:], r_t[:])
                nc.vector.tensor_mul(ot[hp][po : po + 64, :], ps_pv_t[0:64, :], rb_t[:])

            # ---- D: output projection -------------------------------------------
            for qs in range(4):
                for half in range(2):
                    ps_y = ps_sm.tile([128, 512], FP, tag="ps_sm")
                    for ft in range(FT):
                        nc.tensor.matmul(
                            ps_y[:],
                            ot[ft][:, qs * 128 : (qs + 1) * 128],
                            wo_sb[:, ft, half * 512 : (half + 1) * 512],
                            start=(ft == 0),
                            stop=(ft == FT - 1),
                        )
                    y_t = yp.tile([128, 512], FP, tag="y")
                    nc.vector.tensor_tensor(
                        y_t[:], ps_y[:], bo_bc[:, half * 512 : (half + 1) * 512], mybir.AluOpType.add
                    )
                    nc.sync.dma_start(
                        y[c * 512 + qs * 128 : c * 512 + (qs + 1) * 128, half * 512 : (half + 1) * 512],
                        y_t[:],
                    )


_NC_CACHE: dict = {}


def build_nc(T: int = T_FULL):
    if T not in _NC_CACHE:
        nc = bacc.Bacc("TRN2", target_bir_lowering=False, debug=False, num_devices=N_CORES)
        _emit(nc, T)
        nc.compile()
        _NC_CACHE[T] = nc
    return _NC_CACHE[T]


def make_in_maps(x, Wqkv, bqkv, Wo, bo, T: int = T_FULL):
    """Shard full inputs into the 8 per-core input maps."""
    x = np.asarray(x, dtype=np.float32)
    Wqkv = np.asarray(Wqkv, dtype=np.float32)
    bqkv = np.asarray(bqkv, dtype=np.float32)
    Wo = np.asarray(Wo, dtype=np.float32)
    bo = np.asarray(bo, dtype=np.float32)
    zeros_c = np.zeros(C, dtype=np.float32)
    in_maps = []
    for core in range(N_CORES):
        b, g = core // HG, core % HG
        sl = slice(g * F, (g + 1) * F)
        in_maps.append(
            {
                "x": np.ascontiguousarray(x[b, :T]),
                "wq": np.ascontiguousarray(Wqkv[:, sl]) * np.float32(0.125),
                "wk": np.ascontiguousarray(Wqkv[:, C + g * F : C + (g + 1) * F]),
                "wv": np.ascontiguousarray(Wqkv[:, 2 * C + g * F : 2 * C + (g + 1) * F]),
                "bq": np.ascontiguousarray(bqkv[sl]) * np.float32(0.125),
                "bk": np.ascontiguousarray(bqkv[C + g * F : C + (g + 1) * F]),
                "bv": np.ascontiguousarray(bqkv[2 * C + g * F : 2 * C + (g + 1) * F]),
                "wo": np.ascontiguousarray(Wo[sl, :]),
                "bo": bo if g == 0 else zeros_c,
            }
        )
    return in_maps


def kernel(x, Wqkv, bqkv, Wo, bo):
    nc = build_nc(T_FULL)
    in_maps = make_in_maps(x, Wqkv, bqkv, Wo, bo)
    res = run_bass_kernel_spmd(nc, in_maps, core_ids=list(range(N_CORES)))
    out = np.empty((B, T_FULL, C), dtype=np.float32)
    for b in range(B):
        out[b] = res.results[HG * b]["y"] + res.results[HG * b + 1]["y"]
    return out


# revision 6
# speedup vs baseline: 1.4438x; 1.4438x over previous
"""Causal self-attention (B=4, T=2048, C=1024, H=16, D=64) on 8 NeuronCores.

Sharding: core = (batch b, head-group g) with b = core//2, g = core%2.
Each core computes its batch's attention for 8 heads (g picks heads 8g..8g+7)
plus the corresponding slice of the QKV/output projections (tensor parallel,
column/row split).  The output projection is row-parallel, so the full output
for batch b is the SUM of the two partial outputs of cores (2b, 2b+1); that
reduction is done on the host during the gather/unshard step.

Device kernel strategy (per core):
  - x^T materialized chunk-by-chunk via PE transposes (fp32 has no DMA
    transpose).
  - q^T, k^T computed as W^T @ x^T (so no transpose of activations needed);
    v computed in natural [T, D] layout as x @ Wv.  1/sqrt(D) is folded into
    Wq/bq on the host (exact: power of two).
  - scores are computed TRANSPOSED (k-position on partitions) so that the
    probs @ v contraction needs no transpose;  softmax runs without max
    subtraction (scores are bounded ~|2|, exp is safe) and the denominator
    comes free as a 65th "ones" column in the PV matmul.
  - causality by restricting matmul column ranges per k-tile + one 128x128
    triangle mask multiply per diagonal block.
  - all matmuls in float16 (1 cycle/row on PE + fast weight load; 10-bit
    mantissa inputs, fp32 PSUM accumulation).
"""

import os
import tempfile
from contextlib import ExitStack

import numpy as np

import concourse.bass as bass
import concourse.mybir as mybir
import concourse.tile as tile
from concourse import bacc
from concourse.bass_utils import run_bass_kernel_spmd
from concourse.masks import make_identity, make_upper_triangular

B, T_FULL, C, H, D = 4, 2048, 1024, 16, 64
HG = 2                # head-group (tensor-parallel) factor
GH = H // HG          # heads per core = 8
F = GH * D            # per-core projection width = 512
N_CORES = B * HG      # 8
FP = mybir.dt.float32
FPH = mybir.dt.float16


def _emit(nc: bass.Bass, T: int):
    CH = T // 512            # 512-row query chunks
    KO = C // 128            # contraction subtiles for C (8)
    FT = F // 128            # feature tiles (4)
    AF = mybir.ActivationFunctionType

    x = nc.dram_tensor("x", [T, C], FP, kind="ExternalInput").ap()
    wq = nc.dram_tensor("wq", [C, F], FPH, kind="ExternalInput").ap()
    wk = nc.dram_tensor("wk", [C, F], FPH, kind="ExternalInput").ap()
    wv = nc.dram_tensor("wv", [C, F], FPH, kind="ExternalInput").ap()
    bq = nc.dram_tensor("bq", [F], FP, kind="ExternalInput").ap()
    bk = nc.dram_tensor("bk", [F], FP, kind="ExternalInput").ap()
    bv = nc.dram_tensor("bv", [F], FP, kind="ExternalInput").ap()
    wo = nc.dram_tensor("wo", [F, C], FPH, kind="ExternalInput").ap()
    bo = nc.dram_tensor("bo", [C], FP, kind="ExternalInput").ap()
    y = nc.dram_tensor("y", [T, C], FP, kind="ExternalOutput").ap()

    with tile.TileContext(nc) as tc, ExitStack() as ctx:
        const = ctx.enter_context(tc.tile_pool(name="const", bufs=1))
        pers = ctx.enter_context(tc.tile_pool(name="pers", bufs=1))
        xp = ctx.enter_context(tc.tile_pool(name="xp", bufs=4))
        xtp = ctx.enter_context(tc.tile_pool(name="xtp", bufs=1))
        qtp = ctx.enter_context(tc.tile_pool(name="qtp", bufs=1))
        ptp = ctx.enter_context(tc.tile_pool(name="ptp", bufs=2))
        otp = ctx.enter_context(tc.tile_pool(name="otp", bufs=4))
        yp = ctx.enter_context(tc.tile_pool(name="yp", bufs=2))
        smp = ctx.enter_context(tc.tile_pool(name="smp", bufs=2))
        ps_big = ctx.enter_context(tc.tile_pool(name="ps_big", bufs=2, space="PSUM"))
        ps_pv = ctx.enter_context(tc.tile_pool(name="ps_pv", bufs=2, space="PSUM"))
        ps_sm = ctx.enter_context(tc.tile_pool(name="ps_sm", bufs=2, space="PSUM"))

        # ---- constants / weights -------------------------------------------------
        ident = const.tile([128, 128], FP)
        make_identity(nc, ident[:])
        tri = const.tile([128, 128], FPH)       # tri[r, c] = 1.0 if c >= r else 0
        make_upper_triangular(nc, tri[:], val=1.0, diag=True)
        ones128 = const.tile([128, 128], FP)
        nc.gpsimd.memset(ones128[:], 1.0)

        wq_sb = const.tile([128, KO, F], FPH)
        nc.sync.dma_start(wq_sb[:], wq.rearrange("(ko p) f -> p ko f", p=128))
        wk_sb = const.tile([128, KO, F], FPH)
        nc.sync.dma_start(wk_sb[:], wk.rearrange("(ko p) f -> p ko f", p=128))
        wv_sb = const.tile([128, KO, F], FPH)
        nc.sync.dma_start(wv_sb[:], wv.rearrange("(ko p) f -> p ko f", p=128))
        wo_sb = const.tile([128, FT, C], FPH)
        nc.sync.dma_start(wo_sb[:], wo.rearrange("(ft p) c -> p ft c", p=128))

        bq_sb = const.tile([128, FT], FP)
        nc.sync.dma_start(bq_sb[:], bq.rearrange("(ft p) -> p ft", p=128))
        bk_sb = const.tile([128, FT], FP)
        nc.sync.dma_start(bk_sb[:], bk.rearrange("(ft p) -> p ft", p=128))

        bv_bc = const.tile([128, F], FP)
        nc.sync.dma_start(bv_bc[0:1, :], bv.rearrange("(o f) -> o f", o=1))
        nc.gpsimd.partition_broadcast(bv_bc[:], bv_bc[0:1, :])
        bo_bc = const.tile([128, C], FP)
        nc.sync.dma_start(bo_bc[0:1, :], bo.rearrange("(o c) -> o c", o=1))
        nc.gpsimd.partition_broadcast(bo_bc[:], bo_bc[0:1, :])

        # ---- persistent k^T / v -------------------------------------------------
        # k^T head-pair packed: head h lives at partitions 64*(h%2)..+64, pair h//2
        kt_sb = pers.tile([128, FT, T], FPH)
        # v natural, per k-tile, per head, 64 features + ones column
        v_sb = pers.tile([128, T // 128, GH, D + 1], FPH)
        nc.vector.tensor_copy(
            v_sb[:, :, :, D : D + 1],
            ones128[:, 0 : (T // 128) * GH].rearrange("p (k h o) -> p k h o", k=T // 128, o=1),
        )

        for c in range(CH):
            # ---- A: x^T for this chunk ------------------------------------------
            xnat = [xp.tile([128, C], FP, tag="xnat", name=f"xnat_{c}_{i}") for i in range(4)]
            for s in range(4):
                nc.sync.dma_start(xnat[s][:], x[c * 512 + s * 128 : c * 512 + (s + 1) * 128, :])
            xt_sb = xtp.tile([128, KO, 512], FPH)
            for kop in range(KO // 2):
                ps_t = ps_big.tile([128, 1024], FP, tag="ps_big")
                for u in range(2):
                    ko = kop * 2 + u
                    for s in range(4):
                        nc.tensor.transpose(
                            ps_t[:, u * 512 + s * 128 : u * 512 + (s + 1) * 128],
                            xnat[s][:, ko * 128 : (ko + 1) * 128],
                            ident[:],
                        )
                nc.vector.tensor_copy(xt_sb[:, kop * 2 : kop * 2 + 2, :], ps_t[:].rearrange("p (u t) -> p u t", u=2))

            # ---- B: q^T, k^T, v projections -------------------------------------
            qt_sb = qtp.tile([128, FT, 512], FPH)
            for ft in range(FT):
                ps_q = ps_sm.tile([128, 512], FP, tag="ps_sm")
                for ko in range(KO):
                    nc.tensor.matmul(
                        ps_q[:],
                        wq_sb[:, ko, ft * 128 : (ft + 1) * 128],
                        xt_sb[:, ko, :],
                        start=(ko == 0),
                        stop=(ko == KO - 1),
                    )
                nc.vector.tensor_scalar_add(qt_sb[:, ft, :], ps_q[:], bq_sb[:, ft : ft + 1])
            for ft in range(FT):
                ps_k = ps_sm.tile([128, 512], FP, tag="ps_sm")
                for ko in range(KO):
                    nc.tensor.matmul(
                        ps_k[:],
                        wk_sb[:, ko, ft * 128 : (ft + 1) * 128],
                        xt_sb[:, ko, :],
                        start=(ko == 0),
                        stop=(ko == KO - 1),
                    )
                nc.vector.tensor_scalar_add(
                    kt_sb[:, ft, c * 512 : (c + 1) * 512], ps_k[:], bk_sb[:, ft : ft + 1]
                )
            for s in range(4):
                ps_v = ps_sm.tile([128, 512], FP, tag="ps_sm")
                for ko in range(KO):
                    nc.tensor.matmul(
                        ps_v[:],
                        xt_sb[:, ko, s * 128 : (s + 1) * 128],
                        wv_sb[:, ko, :],
                        start=(ko == 0),
                        stop=(ko == KO - 1),
                    )
                nc.vector.tensor_tensor(
                    v_sb[:, c * 4 + s, :, 0:D],
                    ps_v[:].rearrange("p (h d) -> p h d", h=GH),
                    bv_bc[:].rearrange("p (h d) -> p h d", h=GH),
                    mybir.AluOpType.add,
                )

            # ---- C: attention ----------------------------------------------------
            KT = 4 * (c + 1)
            ot = [otp.tile([128, 512], FPH, tag="ot", name=f"ot_{c}_{i}") for i in range(FT)]
            for h in range(GH):
                hp, po = h // 2, 64 * (h % 2)
                ps_pv_t = ps_pv.tile([65, 512], FP, tag="ps_pv")
                for jj in range(0, KT, 2):
                    ps_s = ps_big.tile([128, 1024], FP, tag="ps_big")
                    pt_t = ptp.tile([128, 1024], FPH, tag="pt")
                    regions = []
                    for u in range(2):
                        j = jj + u
                        if j >= KT:
                            break
                        off = max(0, (j - 4 * c) * 128)
                        regions.append((u, j, off))
                        nc.tensor.matmul(
                            ps_s[:, u * 512 + off : (u + 1) * 512],
                            kt_sb[po : po + 64, hp, j * 128 : (j + 1) * 128],
                            qt_sb[po : po + 64, hp, off:512],
                            start=True,
                            stop=True,
                        )
                    if len(regions) == 2 and regions[0][2] == 0 and regions[1][2] == 0:
                        nc.scalar.activation(pt_t[:], ps_s[:], AF.Exp)
                    else:
                        for u, j, off in regions:
                            nc.scalar.activation(
                                pt_t[:, u * 512 + off : (u + 1) * 512],
                                ps_s[:, u * 512 + off : (u + 1) * 512],
                                AF.Exp,
                            )
                    for u, j, off in regions:
                        if j >= 4 * c:  # diagonal block: strict lower triangle -> 0
                            blk = slice(u * 512 + off, u * 512 + off + 128)
                            nc.vector.tensor_mul(pt_t[:, blk], pt_t[:, blk], tri[:])
                    for u, j, off in regions:
                        nc.tensor.matmul(
                            ps_pv_t[:, off:512],
                            v_sb[:, j, h, :],
                            pt_t[:, u * 512 + off : (u + 1) * 512],
                            start=(j == 0),
                            stop=(j == KT - 1),
                        )
                r_t = smp.tile([1, 512], FP, tag="r")
                sum_t = smp.tile([1, 512], FP, tag="sum")
                nc.vector.tensor_copy(sum_t[:], ps_pv_t[64:65, :])
                nc.vector.reciprocal_approx_fast(out=r_t[:], in_=sum_t[:])
                rb_t = smp.tile([64, 512], FP, tag="rb")
                nc.gpsimd.partition_broadcast(rb_t[:], r_t[:])
                nc.vector.tensor_mul(ot[hp][po : po + 64, :], ps_pv_t[0:64, :], rb_t[:])

            # ---- D: output projection -------------------------------------------
            for qs in range(4):
                for half in range(2):
                    ps_y = ps_sm.tile([128, 512], FP, tag="ps_sm")
                    for ft in range(FT):
                        nc.tensor.matmul(
                            ps_y[:],
                            ot[ft][:, qs * 128 : (qs + 1) * 128],
                            wo_sb[:, ft, half * 512 : (half + 1) * 512],
                            start=(ft == 0),
                            stop=(ft == FT - 1),
                        )
                    y_t = yp.tile([128, 512], FP, tag="y")
                    nc.vector.tensor_tensor(
                        y_t[:], ps_y[:], bo_bc[:, half * 512 : (half + 1) * 512], mybir.AluOpType.add
                    )
                    nc.sync.dma_start(
                        y[c * 512 + qs * 128 : c * 512 + (qs + 1) * 128, half * 512 : (half + 1) * 512],
                        y_t[:],
                    )


_NC_CACHE: dict = {}


def build_nc(T: int = T_FULL):
    if T not in _NC_CACHE:
        nc = bacc.Bacc("TRN2", target_bir_lowering=False, debug=False, num_devices=N_CORES)
        _emit(nc, T)
        nc.compile()
        _NC_CACHE[T] = nc
    return _NC_CACHE[T]


def make_in_maps(x, Wqkv, bqkv, Wo, bo, T: int = T_FULL):
    """Shard full inputs into the 8 per-core input maps."""
    x = np.asarray(x, dtype=np.float32)
    Wqkv = np.asarray(Wqkv, dtype=np.float32)
    bqkv = np.asarray(bqkv, dtype=np.float32)
    Wo = np.asarray(Wo, dtype=np.float32)
    bo = np.asarray(bo, dtype=np.float32)
    zeros_c = np.zeros(C, dtype=np.float32)
    in_maps = []
    for core in range(N_CORES):
        b, g = core // HG, core % HG
        sl = slice(g * F, (g + 1) * F)
        in_maps.append(
            {
                "x": np.ascontiguousarray(x[b, :T]),
                "wq": (np.ascontiguousarray(Wqkv[:, sl]) * np.float32(0.125)).astype(np.float16),
                "wk": np.ascontiguousarray(Wqkv[:, C + g * F : C + (g + 1) * F]).astype(np.float16),
                "wv": np.ascontiguousarray(Wqkv[:, 2 * C + g * F : 2 * C + (g + 1) * F]).astype(np.float16),
                "bq": np.ascontiguousarray(bqkv[sl]) * np.float32(0.125),
                "bk": np.ascontiguousarray(bqkv[C + g * F : C + (g + 1) * F]),
                "bv": np.ascontiguousarray(bqkv[2 * C + g * F : 2 * C + (g + 1) * F]),
                "wo": np.ascontiguousarray(Wo[sl, :]).astype(np.float16),
                "bo": bo if g == 0 else zeros_c,
            }
        )
    return in_maps


def kernel(x, Wqkv, bqkv, Wo, bo):
    nc = build_nc(T_FULL)
    in_maps = make_in_maps(x, Wqkv, bqkv, Wo, bo)
    res = run_bass_kernel_spmd(nc, in_maps, core_ids=list(range(N_CORES)))
    out = np.empty((B, T_FULL, C), dtype=np.float32)
    for b in range(B):
        out[b] = res.results[HG * b]["y"] + res.results[HG * b + 1]["y"]
    return out


# revision 9
# speedup vs baseline: 1.5133x; 1.0482x over previous
"""Causal self-attention (B=4, T=2048, C=1024, H=16, D=64) on 8 NeuronCores.

Sharding: core = (batch b, head-group g) with b = core//2, g = core%2.
Each core computes its batch's attention for 8 heads (g picks heads 8g..8g+7)
plus the corresponding slice of the QKV/output projections (tensor parallel,
column/row split).  The output projection is row-parallel, so the full output
for batch b is the SUM of the two partial outputs of cores (2b, 2b+1); that
reduction is done on the host during the gather/unshard step.

Device kernel strategy (per core):
  - x^T materialized chunk-by-chunk via PE transposes (fp32 has no DMA
    transpose).
  - q^T, k^T computed as W^T @ x^T (so no transpose of activations needed);
    v computed in natural [T, D] layout as x @ Wv.  1/sqrt(D) is folded into
    Wq/bq on the host (exact: power of two).
  - scores are computed TRANSPOSED (k-position on partitions) so that the
    probs @ v contraction needs no transpose;  softmax runs without max
    subtraction (scores are bounded ~|2|, exp is safe) and the denominator
    comes free as a 65th "ones" column in the PV matmul.
  - causality by restricting matmul column ranges per k-tile + one 128x128
    triangle mask multiply per diagonal block.
  - all matmuls in float16 (1 cycle/row on PE + fast weight load; 10-bit
    mantissa inputs, fp32 PSUM accumulation).
"""

import os
import tempfile
from contextlib import ExitStack

import numpy as np

import concourse.bass as bass
import concourse.mybir as mybir
import concourse.tile as tile
from concourse import bacc
from concourse.bass_utils import run_bass_kernel_spmd
from concourse.masks import make_identity, make_upper_triangular

B, T_FULL, C, H, D = 4, 2048, 1024, 16, 64
HG = 2                # head-group (tensor-parallel) factor
GH = H // HG          # heads per core = 8
F = GH * D            # per-core projection width = 512
N_CORES = B * HG      # 8
FP = mybir.dt.float32
FPH = mybir.dt.float16


def _emit(nc: bass.Bass, T: int):
    CH = T // 512            # 512-row query chunks
    KO = C // 128            # contraction subtiles for C (8)
    FT = F // 128            # feature tiles (4)
    AF = mybir.ActivationFunctionType

    x = nc.dram_tensor("x", [T, C], FP, kind="ExternalInput").ap()
    wq = nc.dram_tensor("wq", [C, F], FPH, kind="ExternalInput").ap()
    wk = nc.dram_tensor("wk", [C, F], FPH, kind="ExternalInput").ap()
    wv = nc.dram_tensor("wv", [C, F], FPH, kind="ExternalInput").ap()
    bq = nc.dram_tensor("bq", [F], FP, kind="ExternalInput").ap()
    bk = nc.dram_tensor("bk", [F], FP, kind="ExternalInput").ap()
    bv = nc.dram_tensor("bv", [F], FP, kind="ExternalInput").ap()
    wo = nc.dram_tensor("wo", [F, C], FPH, kind="ExternalInput").ap()
    bo = nc.dram_tensor("bo", [C], FP, kind="ExternalInput").ap()
    y = nc.dram_tensor("y", [T, C], FP, kind="ExternalOutput").ap()

    with tile.TileContext(nc) as tc, ExitStack() as ctx:
        const = ctx.enter_context(tc.tile_pool(name="const", bufs=1))
        pers = ctx.enter_context(tc.tile_pool(name="pers", bufs=1))
        xp = ctx.enter_context(tc.tile_pool(name="xp", bufs=6))
        xtp = ctx.enter_context(tc.tile_pool(name="xtp", bufs=2))
        qtp = ctx.enter_context(tc.tile_pool(name="qtp", bufs=2))
        ptp = ctx.enter_context(tc.tile_pool(name="ptp", bufs=4))
        otp = ctx.enter_context(tc.tile_pool(name="otp", bufs=6))
        oup = ctx.enter_context(tc.tile_pool(name="oup", bufs=4))
        yp = ctx.enter_context(tc.tile_pool(name="yp", bufs=3))
        smp = ctx.enter_context(tc.tile_pool(name="smp", bufs=3))
        ps_big = ctx.enter_context(tc.tile_pool(name="ps_big", bufs=2, space="PSUM"))
        ps_pv = ctx.enter_context(tc.tile_pool(name="ps_pv", bufs=2, space="PSUM"))
        ps_sm = ctx.enter_context(tc.tile_pool(name="ps_sm", bufs=2, space="PSUM"))

        # ---- constants / weights ----------------------------------------
        ident = const.tile([128, 128], FP)
        make_identity(nc, ident[:])
        tri = const.tile([128, 128], FPH)       # tri[r, c] = 1.0 if c >= r else 0
        make_upper_triangular(nc, tri[:], val=1.0, diag=True)
        ones128 = const.tile([128, 128], FP)
        nc.gpsimd.memset(ones128[:], 1.0)

        wq_sb = const.tile([128, KO, F], FPH)
        nc.sync.dma_start(wq_sb[:], wq.rearrange("(ko p) f -> p ko f", p=128))
        wk_sb = const.tile([128, KO, F], FPH)
        nc.sync.dma_start(wk_sb[:], wk.rearrange("(ko p) f -> p ko f", p=128))
        wv_sb = const.tile([128, KO, F], FPH)
        nc.sync.dma_start(wv_sb[:], wv.rearrange("(ko p) f -> p ko f", p=128))
        wo_sb = const.tile([128, FT, C], FPH)
        nc.sync.dma_start(wo_sb[:], wo.rearrange("(ft p) c -> p ft c", p=128))

        bq_sb = const.tile([128, FT], FP)
        nc.sync.dma_start(bq_sb[:], bq.rearrange("(ft p) -> p ft", p=128))
        bk_sb = const.tile([128, FT], FP)
        nc.sync.dma_start(bk_sb[:], bk.rearrange("(ft p) -> p ft", p=128))

        bv_bc = const.tile([128, F], FP)
        nc.sync.dma_start(bv_bc[0:1, :], bv.rearrange("(o f) -> o f", o=1))
        nc.gpsimd.partition_broadcast(bv_bc[:], bv_bc[0:1, :])
        bo_bc = const.tile([128, C], FP)
        nc.sync.dma_start(bo_bc[0:1, :], bo.rearrange("(o c) -> o c", o=1))
        nc.gpsimd.partition_broadcast(bo_bc[:], bo_bc[0:1, :])

        # ---- persistent k^T / v (one tile per 512-chunk for precise deps)
        kt_c = []
        v_c = []
        for c in range(CH):
            kt_t = pers.tile([128, FT, 512], FPH, name=f"kt_{c}")
            kt_c.append(kt_t)
            v_t = pers.tile([128, 4, GH, D + 1], FPH, name=f"v_{c}")
            v_c.append(v_t)
            nc.vector.tensor_copy(
                v_t[:, :, :, D : D + 1],
                ones128[:, 0 : 4 * GH].rearrange("p (k h o) -> p k h o", k=4, o=1),
            )

        def kt_at(j):           # lhsT [64, 128] for k-tile j, partition offset po
            return kt_c[j // 4]

        for c in range(CH):
            # ---- A: x^T for this chunk ----------------------------------
            xnat = [xp.tile([128, C], FP, tag="xnat", name=f"xnat_{c}_{i}") for i in range(4)]
            for s in range(4):
                nc.sync.dma_start(xnat[s][:], x[c * 512 + s * 128 : c * 512 + (s + 1) * 128, :])
            xt_sb = xtp.tile([128, KO, 512], FPH)
            for kop in range(KO // 2):
                ps_t = ps_big.tile([128, 1024], FP, tag="ps_big")
                for u in range(2):
                    ko = kop * 2 + u
                    for s in range(4):
                        nc.tensor.transpose(
                            ps_t[:, u * 512 + s * 128 : u * 512 + (s + 1) * 128],
                            xnat[s][:, ko * 128 : (ko + 1) * 128],
                            ident[:],
                        )
                nc.vector.tensor_copy(xt_sb[:, kop * 2 : kop * 2 + 2, :], ps_t[:].rearrange("p (u t) -> p u t", u=2))

            # ---- B: q^T, k^T, v projections ------------------------------
            qt_sb = qtp.tile([128, FT, 512], FPH)
            for ft in range(FT):
                ps_q = ps_sm.tile([128, 512], FP, tag="ps_sm")
                for ko in range(KO):
                    nc.tensor.matmul(
                        ps_q[:],
                        wq_sb[:, ko, ft * 128 : (ft + 1) * 128],
                        xt_sb[:, ko, :],
                        start=(ko == 0),
                        stop=(ko == KO - 1),
                    )
                nc.vector.tensor_scalar_add(qt_sb[:, ft, :], ps_q[:], bq_sb[:, ft : ft + 1])
            for ft in range(FT):
                ps_k = ps_sm.tile([128, 512], FP, tag="ps_sm")
                for ko in range(KO):
                    nc.tensor.matmul(
                        ps_k[:],
                        wk_sb[:, ko, ft * 128 : (ft + 1) * 128],
                        xt_sb[:, ko, :],
                        start=(ko == 0),
                        stop=(ko == KO - 1),
                    )
                nc.vector.tensor_scalar_add(kt_c[c][:, ft, :], ps_k[:], bk_sb[:, ft : ft + 1])
            for s in range(4):
                ps_v = ps_sm.tile([128, 512], FP, tag="ps_sm")
                for ko in range(KO):
                    nc.tensor.matmul(
                        ps_v[:],
                        xt_sb[:, ko, s * 128 : (s + 1) * 128],
                        wv_sb[:, ko, :],
                        start=(ko == 0),
                        stop=(ko == KO - 1),
                    )
                nc.vector.tensor_tensor(
                    v_c[c][:, s, :, 0:D],
                    ps_v[:].rearrange("p (h d) -> p h d", h=GH),
                    bv_bc[:].rearrange("p (h d) -> p h d", h=GH),
                    mybir.AluOpType.add,
                )

            # ---- C: attention, head pairs packed in PE row groups --------
            KT = 4 * (c + 1)
            ot = [otp.tile([128, 512], FPH, tag="ot", name=f"ot_{c}_{i}") for i in range(FT)]
            for hp in range(FT):
                pv_ab = []
                for half, po in ((0, 0), (1, 64)):
                    pv_t = ps_pv.tile([65, 512], FP, tag="ps_pv", name=f"pv_{c}_{hp}_{half}")
                    pv_ab.append(pv_t)
                for j in range(KT):
                    off = max(0, (j - 4 * c) * 128)
                    jc, js = j // 4, (j % 4) * 128
                    ps_s = ps_big.tile([128, 1024], FP, tag="ps_big", name=f"ps_s_{c}_{hp}_{j}")
                    pt_t = ptp.tile([128, 1024], FPH, tag="pt", name=f"pt_{c}_{hp}_{j}")
                    for half, po in ((0, 0), (1, 64)):
                        nc.tensor.matmul(
                            ps_s[:, half * 512 + off : (half + 1) * 512],
                            kt_c[jc][po : po + 64, hp, js : js + 128],
                            qt_sb[po : po + 64, hp, off:512],
                            start=True,
                            stop=True,
                            tile_position=(po, 0),
                        )
                    if off == 0:
                        nc.scalar.activation(pt_t[:], ps_s[:], AF.Exp)
                    else:
                        for half in range(2):
                            nc.scalar.activation(
                                pt_t[:, half * 512 + off : (half + 1) * 512],
                                ps_s[:, half * 512 + off : (half + 1) * 512],
                                AF.Exp,
                            )
                    if j >= 4 * c:
                        for half in range(2):
                            blk = slice(half * 512 + off, half * 512 + off + 128)
                            nc.vector.tensor_mul(pt_t[:, blk], pt_t[:, blk], tri[:])
                    for half in range(2):
                        nc.tensor.matmul(
                            pv_ab[half][:, off:512],
                            v_c[jc][:, j % 4, 2 * hp + half, :],
                            pt_t[:, half * 512 + off : (half + 1) * 512],
                            start=(j == 0),
                            stop=(j == KT - 1),
                        )
                for half, po in ((0, 0), (1, 64)):
                    ou_t = oup.tile([64, 512], FP, tag="ou", name=f"ou_{c}_{hp}_{half}")
                    nc.vector.tensor_copy(ou_t[:], pv_ab[half][0:64, :])
                    sum_t = smp.tile([1, 512], FP, tag="sum", name=f"sum_{c}_{hp}_{half}")
                    nc.vector.tensor_copy(sum_t[:], pv_ab[half][64:65, :])
                    r_t = smp.tile([1, 512], FP, tag="r", name=f"r_{c}_{hp}_{half}")
                    nc.vector.reciprocal_approx_fast(out=r_t[:], in_=sum_t[:])
                    rb_t = smp.tile([64, 512], FP, tag="rb", name=f"rb_{c}_{hp}_{half}")
                    nc.gpsimd.partition_broadcast(rb_t[:], r_t[:])
                    nc.vector.tensor_mul(ot[hp][po : po + 64, :], ou_t[:], rb_t[:])

            # ---- D: output projection ------------------------------------
            for qs in range(4):
                for half in range(2):
                    ps_y = ps_sm.tile([128, 512], FP, tag="ps_sm", name=f"ps_y_{c}_{qs}_{half}")
                    for ft in range(FT):
                        nc.tensor.matmul(
                            ps_y[:],
                            ot[ft][:, qs * 128 : (qs + 1) * 128],
                            wo_sb[:, ft, half * 512 : (half + 1) * 512],
                            start=(ft == 0),
                            stop=(ft == FT - 1),
                        )
                    y_t = yp.tile([128, 512], FP, tag="y", name=f"y_{c}_{qs}_{half}")
                    nc.vector.tensor_tensor(
                        y_t[:], ps_y[:], bo_bc[:, half * 512 : (half + 1) * 512], mybir.AluOpType.add
                    )
                    nc.sync.dma_start(
                        y[c * 512 + qs * 128 : c * 512 + (qs + 1) * 128, half * 512 : (half + 1) * 512],
                        y_t[:],
                    )


_NC_CACHE: dict = {}


def build_nc(T: int = T_FULL):
    if T not in _NC_CACHE:
        nc = bacc.Bacc("TRN2", target_bir_lowering=False, debug=False, num_devices=N_CORES)
        _emit(nc, T)
        nc.compile()
        _NC_CACHE[T] = nc
    return _NC_CACHE[T]


def make_in_maps(x, Wqkv, bqkv, Wo, bo, T: int = T_FULL):
    """Shard full inputs into the 8 per-core input maps."""
    x = np.asarray(x, dtype=np.float32)
    Wqkv = np.asarray(Wqkv, dtype=np.float32)
    bqkv = np.asarray(bqkv, dtype=np.float32)
    Wo = np.asarray(Wo, dtype=np.float32)
    bo = np.asarray(bo, dtype=np.float32)
    zeros_c = np.zeros(C, dtype=np.float32)
    in_maps = []
    for core in range(N_CORES):
        b, g = core // HG, core % HG
        sl = slice(g * F, (g + 1) * F)
        in_maps.append(
            {
                "x": np.ascontiguousarray(x[b, :T]),
                "wq": (np.ascontiguousarray(Wqkv[:, sl]) * np.float32(0.125)).astype(np.float16),
                "wk": np.ascontiguousarray(Wqkv[:, C + g * F : C + (g + 1) * F]).astype(np.float16),
                "wv": np.ascontiguousarray(Wqkv[:, 2 * C + g * F : 2 * C + (g + 1) * F]).astype(np.float16),
                "bq": np.ascontiguousarray(bqkv[sl]) * np.float32(0.125),
                "bk": np.ascontiguousarray(bqkv[C + g * F : C + (g + 1) * F]),
                "bv": np.ascontiguousarray(bqkv[2 * C + g * F : 2 * C + (g + 1) * F]),
                "wo": np.ascontiguousarray(Wo[sl, :]).astype(np.float16),
                "bo": bo if g == 0 else zeros_c,
            }
        )
    return in_maps


def kernel(x, Wqkv, bqkv, Wo, bo):
    nc = build_nc(T_FULL)
    in_maps = make_in_maps(x, Wqkv, bqkv, Wo, bo)
    res = run_bass_kernel_spmd(nc, in_maps, core_ids=list(range(N_CORES)))
    out = np.empty((B, T_FULL, C), dtype=np.float32)
    for b in range(B):
        out[b] = res.results[HG * b]["y"] + res.results[HG * b + 1]["y"]
    return out
